# revision 1
# baseline (speedup 1.0000x reference)
"""Bass/Tile Trainium2 kernel for the CAFBlock fusion (nn_CAFBlock).

Strategy: shard the audio channel dim C_a=128 across 8 NeuronCores (16
channels per core).  BatchNorm2d statistics are per-channel -> fully local.
The tiny video branch (gLN over all channels) is computed redundantly on
every core from a replicated copy of v1, so there are no collectives.

Per-core SBUF layout for the big tensors: partition p = b*64 + k where k is
the video-frame index (t = k*8 + r), free dim = (c_local, r, f).  With this
layout the nearest-interpolated v_attn/v_key factors are constant along the
free dim, so each fused output tile needs only per-partition [128,1] scalar
operands:
    out[ns,c] = a1*(Av*attn) + (Bv*attn) + v_key * relu(a1*Ag + Bg)
computed as one ACT/DVE affine op + one scalar_tensor_tensor op.
"""

import numpy as np

import concourse.bass as bass
import concourse.bacc as bacc
import concourse.tile as tile
import concourse.mybir as mybir
from concourse.bass_utils import run_bass_kernel_spmd

F32 = mybir.dt.float32
AF = mybir.ActivationFunctionType
OP = mybir.AluOpType
AX = mybir.AxisListType
MS = bass.MemorySpace

# problem dims (hardcoded per the harness contract)
B, NS, CA, H, T, FQ, TV = 2, 2, 128, 4, 512, 128, 64
NCORE = 8
CL = CA // NCORE            # 16 local channels per core
N = B * NS                  # 4 (b*ns video samples)
RP = T // TV                # 8 (nearest-interp repeat factor)
BN_EPS, GLN_EPS = 1e-5, 1e-8
NBN = float(B * T * FQ)     # 131072 elements per BN channel
NKEY = float(CA * TV)       # 8192 elements per gLN(key) sample
NATT = float(CA * H * TV)   # 32768 elements per gLN(attn) sample
CF = RP * FQ                # 1024 free elements per channel tile
AFREE = CL * CF             # 16384 free elements of resident a1 shard
OFREE = CL * NS * CF        # 32768 free elements of output


def _build():
    """Builds the SPMD Bass program (same program on all 8 cores)."""
    nc = bacc.Bacc("TRN2", target_bir_lowering=False, debug=False)

    d_a1 = nc.dram_tensor("a1s", [128, AFREE], F32, kind="ExternalInput")
    # consts packed host-side into 3 tensors so they land in 3 fast DMAs
    # cb1 [128, 267]: v1f 0:256 | pcol 256:266 | onec 266:267
    # cb2 [16, 290]:  v1l 0:256 | ploc 256:274 | id16 274:290
    # cb3 [1, 224]:   oner 0:128 | prow 128:224
    d_cb1 = nc.dram_tensor("cb1", [128, 267], F32, kind="ExternalInput")
    d_cb2 = nc.dram_tensor("cb2", [CL, 290], F32, kind="ExternalInput")
    d_cb3 = nc.dram_tensor("cb3", [1, 224], F32, kind="ExternalInput")
    d_out = nc.dram_tensor("out", [128, OFREE], F32, kind="ExternalOutput")

    with tile.TileContext(nc) as tc:
        with (
            tc.tile_pool(name="pres", bufs=8) as pres,
            tc.tile_pool(name="pconst", bufs=1) as pc,
            tc.tile_pool(name="pscr", bufs=2) as pscr,
            tc.tile_pool(name="pgate", bufs=4) as pgate,
            tc.tile_pool(name="ps0", bufs=4) as ps0,
            tc.tile_pool(name="ps1", bufs=4) as ps1,
            tc.tile_pool(name="pout", bufs=3) as pout,
            tc.tile_pool(name="pps", bufs=1, space=MS.PSUM) as pps,
            tc.tile_pool(name="ppt", bufs=2, space=MS.PSUM) as ppt,
            tc.tile_pool(name="ppb", bufs=1, space=MS.PSUM) as ppb,
        ):
            # ---------------- constants first (3 fast HWDGE DMAs) ---------
            cb1 = pc.tile([128, 267], F32, tag="cb1")
            cb2 = pc.tile([CL, 290], F32, tag="cb2")
            cb3 = pc.tile([1, 224], F32, tag="cb3")
            nc.sync.dma_start(cb1[:], d_cb1.ap()[:])
            nc.sync.dma_start(cb2[:], d_cb2.ap()[:])
            nc.sync.dma_start(cb3[:], d_cb3.ap()[:])
            v1f = cb1[:, 0:256]
            pcol = cb1[:, 256:266]
            onec = cb1[:, 266:267]
            v1l = cb2[:, 0:256]
            ploc = cb2[:, 256:274]
            id16 = cb2[:, 274:290]
            oner = cb3[:, 0:128]
            prow = cb3[:, 128:224]

            # ---------------- input DMAs (HWDGE, 1MB each) ----------------
            res = []
            for g in range(8):
                t = pres.tile([128, 2048], F32, tag="res")
                nc.sync.dma_start(t[:], d_a1.ap()[:, g * 2048:(g + 1) * 2048])
                res.append(t)

            def a1c(c):
                return res[c // 2][:, (c % 2) * CF:(c % 2) * CF + CF]

            # ---------------- v-branch full-channel stats -----------------
            # key: vk = v1*wk + bk ; per-sample sums over (c, tv)
            vkf = pc.tile([128, N * TV], F32, tag="vkf")
            nc.vector.tensor_scalar(vkf[:], v1f[:], pcol[:, 0:1], pcol[:, 1:2],
                                    OP.mult, OP.add)
            ks = pc.tile([128, 8], F32, tag="ks")
            nc.vector.tensor_reduce(
                ks[:, 0:4], vkf[:].rearrange("p (n t) -> p n t", n=N, t=TV),
                axis=AX.X, op=OP.add)
            scrk = pc.tile([128, N * TV], F32, tag="scrk")
            nc.vector.tensor_tensor(scrk[:], vkf[:], vkf[:], OP.mult)
            nc.vector.tensor_reduce(
                ks[:, 4:8], scrk[:].rearrange("p (n t) -> p n t", n=N, t=TV),
                axis=AX.X, op=OP.add)

            # attn: va[h] = v1*wa_h + ba_h ; per-sample sums over (c, h, tv)
            va = pc.tile([128, H * N * TV], F32, tag="va")
            for h in range(H):
                nc.vector.tensor_scalar(
                    va[:, h * N * TV:(h + 1) * N * TV], v1f[:],
                    pcol[:, 2 + h:3 + h], pcol[:, 6 + h:7 + h], OP.mult, OP.add)
            asum = pc.tile([128, 8], F32, tag="asum")
            nc.vector.tensor_reduce(
                asum[:, 0:4],
                va[:].rearrange("p (h n t) -> p n h t", h=H, n=N, t=TV),
                axis=AX.XY, op=OP.add)
            scra = pc.tile([128, H * N * TV], F32, tag="scra")
            nc.vector.tensor_tensor(scra[:], va[:], va[:], OP.mult)
            nc.vector.tensor_reduce(
                asum[:, 4:8],
                scra[:].rearrange("p (h n t) -> p n h t", h=H, n=N, t=TV),
                axis=AX.XY, op=OP.add)

            # ------- v-branch cross-partition reduction + finalize --------
            # (independent of the BN stats: runs while a1 is still streaming)
            pp_ks = pps.tile([1, 8], F32, tag="ppks")
            pp_as = pps.tile([1, 8], F32, tag="ppas")
            nc.tensor.matmul(pp_ks[:], onec[:], ks[:], start=True, stop=True)
            nc.tensor.matmul(pp_as[:], onec[:], asum[:], start=True, stop=True)
            kr = pc.tile([1, 8], F32, tag="kr")
            ar = pc.tile([1, 8], F32, tag="ar")
            nc.scalar.copy(kr[:], pp_ks[:])
            nc.scalar.copy(ar[:], pp_as[:])

            # v rows: kv layout 0:4 kmean | 4:8 kex2 | 8:12 kvar | 12:16 km^2
            #         16:20 amean | 20:24 aex2 | 24:28 avar | 28:32 am^2
            kv = pc.tile([1, 32], F32, tag="kv")
            nc.vector.tensor_scalar_mul(kv[:, 0:4], kr[:, 0:4], 1.0 / NKEY)
            nc.vector.tensor_scalar_mul(kv[:, 4:8], kr[:, 4:8], 1.0 / NKEY)
            nc.vector.tensor_tensor(kv[:, 12:16], kv[:, 0:4], kv[:, 0:4],
                                    OP.mult)
            nc.vector.tensor_tensor(kv[:, 8:12], kv[:, 4:8], kv[:, 12:16],
                                    OP.subtract)
            nc.vector.tensor_scalar_mul(kv[:, 16:20], ar[:, 0:4], 1.0 / NATT)
            nc.vector.tensor_scalar_mul(kv[:, 20:24], ar[:, 4:8], 1.0 / NATT)
            nc.vector.tensor_tensor(kv[:, 28:32], kv[:, 16:20], kv[:, 16:20],
                                    OP.mult)
            nc.vector.tensor_tensor(kv[:, 24:28], kv[:, 20:24], kv[:, 28:32],
                                    OP.subtract)

            def rsqrt_rows(qa, width, pref):
                # 1/sqrt(q) via exp(-0.5*ln(q)) + one Newton polish
                lnq = pc.tile([1, width], F32, tag=pref + "ln")
                r0 = pc.tile([1, width], F32, tag=pref + "r0")
                rr = pc.tile([1, width], F32, tag=pref + "rr")
                ntt = pc.tile([1, width], F32, tag=pref + "nt")
                nc.scalar.activation(lnq[:], qa, AF.Ln)
                nc.scalar.activation(r0[:], lnq[:], AF.Exp, scale=-0.5)
                nc.vector.tensor_tensor(ntt[:], r0[:], r0[:], OP.mult)
                nc.vector.tensor_tensor(ntt[:], qa, ntt[:], OP.mult)
                nc.vector.tensor_scalar(ntt[:], ntt[:], -1.0, 3.0, OP.mult,
                                        OP.add)
                nc.vector.tensor_scalar_mul(rr[:], r0[:], 0.5)
                nc.vector.tensor_tensor(rr[:], rr[:], ntt[:], OP.mult)
                return rr

            qv = pc.tile([1, 8], F32, tag="qv")
            nc.vector.tensor_scalar_add(qv[:, 0:4], kv[:, 8:12], GLN_EPS)
            nc.vector.tensor_scalar_add(qv[:, 4:8], kv[:, 24:28], GLN_EPS)
            rsv = rsqrt_rows(qv[:], 8, "v")  # 0:4 rs_key | 4:8 rs_attn

            # bc1 row [1,28]: kmean(4) | rs_key(4) | amean*rs_attn(4) |
            #                 rs_attn repeated n-major h-minor (16)
            b1 = pc.tile([1, 28], F32, tag="b1")
            nc.vector.tensor_copy(b1[:, 0:4], kv[:, 0:4])
            nc.vector.tensor_copy(b1[:, 4:8], rsv[:, 0:4])
            nc.vector.tensor_tensor(b1[:, 8:12], kv[:, 16:20], rsv[:, 4:8],
                                    OP.mult)
            b1rep = b1[:, 12:28].rearrange("p (n x) -> p n x", n=N, x=H)
            rsat = rsv[:, 4:8].rearrange("p (n x) -> p n x", n=N, x=1)
            for h in range(H):
                nc.vector.tensor_copy(b1rep[:, :, h:h + 1], rsat[:])

            pp_b1 = ppb.tile([128, 28], F32, tag="ppb1")
            nc.tensor.matmul(pp_b1[:], oner[:], b1[:], start=True, stop=True)
            bc1 = pc.tile([128, 28], F32, tag="bc1")
            nc.scalar.copy(bc1[:], pp_b1[:])

            # ---------------- local v-branch ------------------------------
            # ploc: wk 0 | bk 1 | gk 2 | bek 3 | wa 4:8 | ba 8:12 |
            #       ga/4 12:16 | sum(ga)/4 16 | sum(bea)/4 17
            vkl = pc.tile([CL, N * TV], F32, tag="vkl")
            nc.vector.tensor_scalar(vkl[:], v1l[:], ploc[:, 0:1], ploc[:, 1:2],
                                    OP.mult, OP.add)
            kscol = pc.tile([CL, N], F32, tag="kscol")
            kbcol = pc.tile([CL, N], F32, tag="kbcol")
            for n in range(N):
                nc.vector.tensor_tensor(kscol[:, n:n + 1], ploc[:, 2:3],
                                        bc1[0:CL, 4 + n:5 + n], OP.mult)
                nc.vector.tensor_tensor(kbcol[:, n:n + 1], kscol[:, n:n + 1],
                                        bc1[0:CL, n:n + 1], OP.mult)
                nc.vector.tensor_tensor(kbcol[:, n:n + 1], ploc[:, 3:4],
                                        kbcol[:, n:n + 1], OP.subtract)
            # vkln/soft stored in (ns, b, tv) column order so the transpose
            # lhsT slice [16, 128] is contiguous: perm(n) = (n%2)*2 + n//2
            perm = [(n % 2) * 2 + n // 2 for n in range(N)]
            vkln = pc.tile([CL, N * TV], F32, tag="vkln")
            for n in range(N):
                nc.vector.tensor_scalar(
                    vkln[:, perm[n] * TV:(perm[n] + 1) * TV],
                    vkl[:, n * TV:(n + 1) * TV],
                    kscol[:, n:n + 1], kbcol[:, n:n + 1], OP.mult, OP.add)

            val = pc.tile([CL, H * N * TV], F32, tag="val")
            for h in range(H):
                nc.vector.tensor_scalar(
                    val[:, h * N * TV:(h + 1) * N * TV], v1l[:],
                    ploc[:, 4 + h:5 + h], ploc[:, 8 + h:9 + h], OP.mult, OP.add)
            ga16 = pc.tile([CL, N * H], F32, tag="ga16")
            for n in range(N):
                nc.vector.tensor_copy(ga16[:, n * H:(n + 1) * H],
                                      ploc[:, 12:16])
            sc16 = pc.tile([CL, N * H], F32, tag="sc16")
            nc.vector.tensor_tensor(sc16[:], ga16[:], bc1[0:CL, 12:28],
                                    OP.mult)
            bicol = pc.tile([CL, N], F32, tag="bicol")
            for n in range(N):
                nc.vector.tensor_tensor(bicol[:, n:n + 1], ploc[:, 16:17],
                                        bc1[0:CL, 8 + n:9 + n], OP.mult)
                nc.vector.tensor_tensor(bicol[:, n:n + 1], ploc[:, 17:18],
                                        bicol[:, n:n + 1], OP.subtract)
            # vm[n] = sum_h val[h,n]*sc16[n,h] + bicol[n]  (ga,bea host-/4)
            vm = pc.tile([CL, N * TV], F32, tag="vm")
            for n in range(N):
                dst = vm[:, n * TV:(n + 1) * TV]
                nc.vector.tensor_scalar(
                    dst, val[:, n * TV:n * TV + TV],
                    sc16[:, n * H:n * H + 1], bicol[:, n:n + 1],
                    OP.mult, OP.add)
                for h in range(1, H):
                    nc.vector.scalar_tensor_tensor(
                        dst, val[:, h * N * TV + n * TV:h * N * TV + n * TV + TV],
                        sc16[:, n * H + h:n * H + h + 1], dst, OP.mult, OP.add)
            # softmax over tv per (c, n)
            mx = pc.tile([CL, N], F32, tag="mx")
            nc.vector.tensor_reduce(
                mx[:], vm[:].rearrange("p (n t) -> p n t", n=N, t=TV),
                axis=AX.X, op=OP.max)
            nmx = pc.tile([CL, N], F32, tag="nmx")
            nc.vector.tensor_scalar_mul(nmx[:], mx[:], -1.0)
            ex = pc.tile([CL, N * TV], F32, tag="ex")
            ssum = pc.tile([CL, N], F32, tag="ssum")
            for n in range(N):
                nc.scalar.activation(
                    ex[:, n * TV:(n + 1) * TV], vm[:, n * TV:(n + 1) * TV],
                    AF.Exp, bias=nmx[:, n:n + 1],
                    accum_out=ssum[:, n:n + 1])
            rcp = pc.tile([CL, N], F32, tag="rcp")
            nc.vector.reciprocal(rcp[:], ssum[:])
            soft = pc.tile([CL, N * TV], F32, tag="soft")
            for n in range(N):
                nc.vector.tensor_scalar_mul(
                    soft[:, perm[n] * TV:(perm[n] + 1) * TV],
                    ex[:, n * TV:(n + 1) * TV], rcp[:, n:n + 1])

            # ---------------- transpose to (b,k) x (ns,c) -----------------
            # out[(b,tv), c] = src[c, (2b+ns)*TV + tv] via lhsT^T @ I16 with a
            # strided lhsT view gathering both b halves (M=128, K=16).
            tkey = pc.tile([128, NS * CL], F32, tag="tkey")
            tatt = pc.tile([128, NS * CL], F32, tag="tatt")
            for (src, dst) in ((vkln, tkey), (soft, tatt)):
                for ns in range(NS):
                    pt = ppt.tile([128, CL], F32, tag="tk")
                    nc.tensor.matmul(pt[:], src[:, ns * B * TV:(ns + 1) * B * TV],
                                     id16[:], start=True, stop=True)
                    nc.scalar.copy(dst[:, ns * CL:(ns + 1) * CL], pt[:])

            # ---------------- BN stats (pipelined with input DMA) ---------
            # per-partition sums on DVE (ts + accum_out); sums of squares on
            # ACT (Square + accum_out), which is otherwise idle here
            sums = pc.tile([128, CL], F32, tag="sums")
            sqs = pc.tile([128, CL], F32, tag="sqs")
            for c in range(CL):
                scrd = pscr.tile([128, CF], F32, tag="scrd")
                nc.vector.tensor_scalar(scrd[:], a1c(c), 1.0, None, OP.mult,
                                        OP.add, accum_out=sums[:, c:c + 1])
                scrs = pscr.tile([128, CF], F32, tag="scrs")
                nc.scalar.activation(scrs[:], a1c(c), AF.Square,
                                     accum_out=sqs[:, c:c + 1])

            pp_sm = pps.tile([1, CL], F32, tag="ppsm")
            pp_sq = pps.tile([1, CL], F32, tag="ppsq")
            nc.tensor.matmul(pp_sm[:], onec[:], sums[:], start=True, stop=True)
            nc.tensor.matmul(pp_sq[:], onec[:], sqs[:], start=True, stop=True)
            sm = pc.tile([1, CL], F32, tag="sm")
            sq = pc.tile([1, CL], F32, tag="sq")
            nc.scalar.copy(sm[:], pp_sm[:])
            nc.scalar.copy(sq[:], pp_sq[:])

            # rw layout: 0:16 mean | 16:32 ex2 | 32:48 mts | 48:64 var
            rw = pc.tile([1, 64], F32, tag="rw")
            nc.vector.tensor_scalar_mul(rw[:, 0:16], sm[:], 1.0 / NBN)
            nc.vector.tensor_scalar_mul(rw[:, 16:32], sq[:], 1.0 / NBN)
            nc.vector.tensor_tensor(rw[:, 32:48], rw[:, 0:16], rw[:, 0:16],
                                    OP.mult)
            nc.vector.tensor_tensor(rw[:, 48:64], rw[:, 16:32], rw[:, 32:48],
                                    OP.subtract)

            # qb [1,32]: var*wv^2+eps | var*wg^2+eps
            # prow layout: wv 0:16 | gv 16:32 | bev 32:48 | wg 48:64
            #              gg 64:80 | beg 80:96
            qb = pc.tile([1, 32], F32, tag="qb")
            w2 = pc.tile([1, 32], F32, tag="w2")
            nc.vector.tensor_tensor(w2[:, 0:16], prow[:, 0:16], prow[:, 0:16],
                                    OP.mult)
            nc.vector.tensor_tensor(w2[:, 16:32], prow[:, 48:64],
                                    prow[:, 48:64], OP.mult)
            nc.vector.tensor_tensor(qb[:, 0:16], rw[:, 48:64], w2[:, 0:16],
                                    OP.mult)
            nc.vector.tensor_tensor(qb[:, 16:32], rw[:, 48:64], w2[:, 16:32],
                                    OP.mult)
            nc.vector.tensor_scalar_add(qb[:], qb[:], BN_EPS)
            rsb = rsqrt_rows(qb[:], 32, "b")  # 0:16 val | 16:32 gate

            # Av/Bv/Ag/Bg row [1,64]
            ab = pc.tile([1, 64], F32, tag="ab")
            nc.vector.tensor_tensor(ab[:, 0:16], rsb[:, 0:16], prow[:, 16:32],
                                    OP.mult)
            nc.vector.tensor_tensor(ab[:, 0:16], ab[:, 0:16], prow[:, 0:16],
                                    OP.mult)
            nc.vector.tensor_tensor(ab[:, 16:32], rw[:, 0:16], ab[:, 0:16],
                                    OP.mult)
            nc.vector.tensor_tensor(ab[:, 16:32], prow[:, 32:48], ab[:, 16:32],
                                    OP.subtract)
            nc.vector.tensor_tensor(ab[:, 32:48], rsb[:, 16:32],
                                    prow[:, 64:80], OP.mult)
            nc.vector.tensor_tensor(ab[:, 32:48], ab[:, 32:48], prow[:, 48:64],
                                    OP.mult)
            nc.vector.tensor_tensor(ab[:, 48:64], rw[:, 0:16], ab[:, 32:48],
                                    OP.mult)
            nc.vector.tensor_tensor(ab[:, 48:64], prow[:, 80:96], ab[:, 48:64],
                                    OP.subtract)

            pp_ab = ppb.tile([128, 64], F32, tag="ppab")
            nc.tensor.matmul(pp_ab[:], oner[:], ab[:], start=True, stop=True)
            bcab = pc.tile([128, 64], F32, tag="bcab")
            nc.scalar.copy(bcab[:], pp_ab[:])

            # alpha/beta tiles [128, 32]
            alpha = pc.tile([128, NS * CL], F32, tag="alpha")
            beta = pc.tile([128, NS * CL], F32, tag="beta")
            for ns in range(NS):
                sl = slice(ns * CL, (ns + 1) * CL)
                nc.vector.tensor_tensor(alpha[:, sl], tatt[:, sl],
                                        bcab[:, 0:16], OP.mult)
                nc.vector.tensor_tensor(beta[:, sl], tatt[:, sl],
                                        bcab[:, 16:32], OP.mult)

            # ---------------- fused output loop ---------------------------
            for c in range(CL):
                src = a1c(c)
                gate = pgate.tile([128, CF], F32, tag="gate")
                nc.scalar.activation(gate[:], src, AF.Relu,
                                     bias=bcab[:, 48 + c:49 + c],
                                     scale=bcab[:, 32 + c:33 + c])
                s0 = ps0.tile([128, CF], F32, tag="s0")
                nc.scalar.activation(s0[:], src, AF.Identity,
                                     bias=beta[:, c:c + 1],
                                     scale=alpha[:, c:c + 1])
                s1 = ps1.tile([128, CF], F32, tag="s1")
                if c % 4 != 3:
                    nc.vector.tensor_scalar(s1[:], src,
                                            alpha[:, CL + c:CL + c + 1],
                                            beta[:, CL + c:CL + c + 1],
                                            OP.mult, OP.add)
                else:
                    nc.scalar.activation(s1[:], src, AF.Identity,
                                         bias=beta[:, CL + c:CL + c + 1],
                                         scale=alpha[:, CL + c:CL + c + 1])
                if c % 2 == 0:
                    ost = pout.tile([128, 2 * NS * CF], F32, tag="ost")
                base = (c % 2) * NS * CF
                nc.vector.scalar_tensor_tensor(
                    ost[:, base:base + CF], gate[:], tkey[:, c:c + 1], s0[:],
                    OP.mult, OP.add)
                nc.vector.scalar_tensor_tensor(
                    ost[:, base + CF:base + 2 * CF], gate[:],
                    tkey[:, CL + c:CL + c + 1], s1[:], OP.mult, OP.add)
                if c % 2 == 1:
                    nc.sync.dma_start(
                        d_out.ap()[:, (c - 1) * NS * CF:(c + 1) * NS * CF],
                        ost[:])

    nc.compile()
    return nc


_NC_CACHE = None


def _get_nc():
    global _NC_CACHE
    if _NC_CACHE is None:
        _NC_CACHE = _build()
    return _NC_CACHE


def _pack_inputs(a1, v1, w_gate, b_gate, g_gate, be_gate,
                 w_val, b_val, g_val, be_val,
                 w_attn, b_attn, g_attn, be_attn,
                 w_key, b_key, g_key, be_key):
    f32 = np.float32
    a1 = np.asarray(a1, f32)
    v1 = np.asarray(v1, f32)
    # full-channel tensors (replicated)
    v1f = np.ascontiguousarray(v1.transpose(1, 0, 2).reshape(CA, N * TV))
    wa2 = np.asarray(w_attn, f32).reshape(CA, H)
    ba2 = np.asarray(b_attn, f32).reshape(CA, H)
    ga2 = np.asarray(g_attn, f32).reshape(CA, H)
    bea2 = np.asarray(be_attn, f32).reshape(CA, H)
    pcol = np.concatenate(
        [np.asarray(w_key, f32)[:, None], np.asarray(b_key, f32)[:, None],
         wa2, ba2], axis=1)
    cb1 = np.concatenate([v1f, pcol, np.ones((CA, 1), f32)], axis=1)
    cb1 = np.ascontiguousarray(cb1)
    id16 = np.eye(CL, dtype=f32)

    in_maps = []
    for i in range(NCORE):
        sl = slice(i * CL, (i + 1) * CL)
        x = a1[:, sl].reshape(B, CL, TV, RP, FQ)
        x = np.ascontiguousarray(x.transpose(0, 2, 1, 3, 4))
        a1s = x.reshape(128, AFREE)
        v1l = np.ascontiguousarray(
            v1[:, sl].transpose(1, 0, 2).reshape(CL, N * TV))
        ga4 = ga2[sl] * 0.25
        ploc = np.concatenate(
            [np.asarray(w_key, f32)[sl, None], np.asarray(b_key, f32)[sl, None],
             np.asarray(g_key, f32)[sl, None], np.asarray(be_key, f32)[sl, None],
             wa2[sl], ba2[sl], ga4,
             ga4.sum(1, keepdims=True),
             (bea2[sl] * 0.25).sum(1, keepdims=True)], axis=1)
        cb2 = np.ascontiguousarray(
            np.concatenate([v1l, ploc, id16], axis=1))
        prow = np.concatenate(
            [np.asarray(w_val, f32)[sl], np.asarray(g_val, f32)[sl],
             np.asarray(be_val, f32)[sl], np.asarray(w_gate, f32)[sl],
             np.asarray(g_gate, f32)[sl],
             np.asarray(be_gate, f32)[sl]])[None, :]
        cb3 = np.ascontiguousarray(
            np.concatenate([np.ones((1, 128), f32), prow], axis=1))
        in_maps.append({"a1s": a1s, "cb1": cb1, "cb2": cb2, "cb3": cb3})
    return in_maps


def _unpack_output(results):
    out = np.empty((N, CA, T, FQ), np.float32)
    for i in range(NCORE):
        r = np.asarray(results[i]["out"]).reshape(B, TV, CL, NS, RP, FQ)
        r = r.transpose(0, 3, 2, 1, 4, 5).reshape(N, CL, T, FQ)
        out[:, i * CL:(i + 1) * CL] = r
    return out


def _install_ntff_shim():
    """The agent image's ``antenv`` lacks ``axon_hooks``; recreate it and
    register the ctypes NTFF hook against /opt/axon/libaxon_pjrt.so (the
    same mechanism trn_boot uses when the module exists)."""
    import sys
    import types
    import ctypes
    import contextlib

    if "antenv.axon_hooks" in sys.modules:
        return True
    so_path = "/opt/axon/libaxon_pjrt.so"
    try:
        lib = ctypes.CDLL(so_path)
    except OSError:
        return False
    if not hasattr(lib, "axon_start_nrt_profile"):
        return False
    lib.axon_start_nrt_profile.argtypes = [ctypes.POINTER(ctypes.c_int64),
                                           ctypes.c_size_t]
    lib.axon_start_nrt_profile.restype = ctypes.c_int64
    lib.axon_stop_nrt_profile.argtypes = [ctypes.c_char_p]
    lib.axon_stop_nrt_profile.restype = ctypes.c_int64

    @contextlib.contextmanager
    def _hook(output_dir, device_ids):
        import jax
        jax.devices()
        if device_ids:
            ids = (ctypes.c_int64 * len(device_ids))(*device_ids)
            rc = lib.axon_start_nrt_profile(ids, len(device_ids))
        else:
            rc = lib.axon_start_nrt_profile(None, 0)
        if rc != 0:
            raise RuntimeError(f"axon_start_nrt_profile rc={rc}")
        try:
            yield
        finally:
            n = lib.axon_stop_nrt_profile(str(output_dir).encode())
            print(f"profile: {n} file(s) written to {output_dir}",
                  file=sys.stderr)

    mod = types.ModuleType("antenv.axon_hooks")
    _state = {"hook": _hook}
    mod.get_axon_ntff_profile_hook = lambda: _state["hook"]

    def set_axon_ntff_profile_hook(h):
        _state["hook"] = h

    mod.set_axon_ntff_profile_hook = set_axon_ntff_profile_hook
    import antenv
    antenv.axon_hooks = mod
    sys.modules["antenv.axon_hooks"] = mod
    return True


def run(inputs, trace=False, **trace_kwargs):
    """Returns (output, BassKernelResults)."""
    nc = _get_nc()
    in_maps = _pack_inputs(**inputs)
    if trace and not _install_ntff_shim():
        trace = False
    br = run_bass_kernel_spmd(nc, in_maps, core_ids=list(range(NCORE)),
                              trace=trace, **trace_kwargs)
    return _unpack_output(br.results), br


def kernel(**inputs):
    out, _ = run(inputs)
    return out



# revision 5
# speedup vs baseline: 1.1537x; 1.1537x over previous
"""Bass/Tile Trainium2 kernel for the CAFBlock fusion (nn_CAFBlock).

Strategy: shard the audio channel dim C_a=128 across 8 NeuronCores (16
channels per core).  BatchNorm2d statistics are per-channel -> fully local.
The tiny video branch (gLN over all channels) is computed redundantly on
every core from a replicated copy of v1, so there are no collectives.

Per-core SBUF layout for the big tensors: partition p = b*64 + k where k is
the video-frame index (t = k*8 + r), free dim = (c_local, r, f).  With this
layout the nearest-interpolated v_attn/v_key factors are constant along the
free dim, so the fused output

    out[ns,c] = (attn_ns*Av)ated * src + key_ns * relu(Ag*src+Bg) + attn_ns*Bv

is computed on the *tensor engine* as two accumulated diagonal matmuls per
(ns, half): diag(alpha_ns) @ src + diag(key_ns) @ gate, with the rank-1
beta term folded into the PSUM->SBUF copy as a per-partition bias.  a1 is
shipped to the device in fp16 (halves input DMA traffic; rel err ~5e-4 vs
the 2e-2 gate).  BN statistics come from per-channel bn_stats pairs; all
rsqrts use an integer-Newton iteration on the DVE so the scalar engine
loads exactly one activation table (exp, for the softmax).
"""

import numpy as np

import concourse.bass as bass
import concourse.bacc as bacc
import concourse.tile as tile
import concourse.mybir as mybir
from concourse.bass_utils import run_bass_kernel_spmd

F32 = mybir.dt.float32
FP16 = mybir.dt.float16
I32 = mybir.dt.int32
AF = mybir.ActivationFunctionType
OP = mybir.AluOpType
AX = mybir.AxisListType
MS = bass.MemorySpace

# problem dims (hardcoded per the harness contract)
B, NS, CA, H, T, FQ, TV = 2, 2, 128, 4, 512, 128, 64
NCORE = 8
CL = CA // NCORE            # 16 local channels per core
N = B * NS                  # 4 (b*ns video samples)
RP = T // TV                # 8 (nearest-interp repeat factor)
BN_EPS, GLN_EPS = 1e-5, 1e-8
NBN = float(B * T * FQ)     # 131072 elements per BN channel
NKEY = float(CA * TV)       # 8192 elements per gLN(key) sample
NATT = float(CA * H * TV)   # 32768 elements per gLN(attn) sample
CF = RP * FQ                # 1024 free elements per channel tile
AFREE = CL * CF             # 16384 free elements of resident a1 shard
OFREE = CL * NS * CF        # 32768 free elements of output
GC = 8                      # channels per finalize group
K_MAGIC = 0x5F3759DF

# channels whose second PSUM->SBUF copy runs on ACT instead of DVE
ACT_COPY1 = frozenset((1, 3, 5, 8, 10, 12))


def _rsqrt_hack(nc, pc, q, width, pref):
    """y = 1/sqrt(q) for positive q via int bit-hack + 3 Newton steps.

    Runs entirely on the DVE (no activation tables)."""
    kcol = pc.tile([1, width], I32, tag=pref + "kc")
    t1 = pc.tile([1, width], I32, tag=pref + "t1")
    y = pc.tile([1, width], F32, tag=pref + "y")
    ysq = pc.tile([1, width], F32, tag=pref + "ys")
    nc.vector.memset(kcol[:], K_MAGIC)
    nc.vector.tensor_scalar(t1[:], q.bitcast(I32), 1, None,
                            OP.logical_shift_right)
    nc.vector.tensor_tensor(y[:].bitcast(I32), kcol[:], t1[:], OP.subtract)
    for _ in range(3):
        nc.vector.tensor_tensor(ysq[:], y[:], y[:], OP.mult)
        nc.vector.tensor_tensor(ysq[:], q, ysq[:], OP.mult)
        nc.vector.tensor_scalar(ysq[:], ysq[:], -0.5, 1.5, OP.mult, OP.add)
        nc.vector.tensor_tensor(y[:], y[:], ysq[:], OP.mult)
    return y


def _build():
    """Builds the SPMD Bass program (same program on all 8 cores)."""
    nc = bacc.Bacc("TRN2", target_bir_lowering=False, debug=False)

    d_a1 = nc.dram_tensor("a1s", [128, AFREE], FP16, kind="ExternalInput")
    # consts packed host-side:
    # cb1 [128, 266]: v1f 0:256 | pcol 256:266
    # cb2 [16, 290]:  v1l 0:256 | ploc 256:274 | id16 274:290
    # cb3 [1, 257]:   oner 0:128 | prow 128:256 | onec-col via memset
    # cbh [128, 128] fp16: identity
    d_cb1 = nc.dram_tensor("cb1", [128, 266], F32, kind="ExternalInput")
    d_cb2 = nc.dram_tensor("cb2", [CL, 290], F32, kind="ExternalInput")
    d_cb3 = nc.dram_tensor("cb3", [1, 256], F32, kind="ExternalInput")
    d_cbh = nc.dram_tensor("cbh", [128, 128], FP16, kind="ExternalInput")
    d_out = nc.dram_tensor("out", [128, OFREE], F32, kind="ExternalOutput")

    with tile.TileContext(nc) as tc:
        with (
            tc.tile_pool(name="pout", bufs=3, space=MS.PSUM) as ppo,
            tc.tile_pool(name="psmall", bufs=2, space=MS.PSUM) as pps,
            tc.tile_pool(name="pres", bufs=8) as pres,
            tc.tile_pool(name="pconst", bufs=1) as pc,
            tc.tile_pool(name="pgate", bufs=3) as pgate,
            tc.tile_pool(name="pdiag", bufs=8) as pdiag,
            tc.tile_pool(name="post", bufs=3) as post,
        ):
            # ---------------- constants (4 fast HWDGE DMAs) ---------------
            cb1 = pc.tile([128, 266], F32, tag="cb1")
            cb2 = pc.tile([CL, 290], F32, tag="cb2")
            cb3 = pc.tile([1, 256], F32, tag="cb3")
            idh = pc.tile([128, 128], FP16, tag="idh")
            nc.sync.dma_start(cb1[:], d_cb1.ap()[:])
            nc.sync.dma_start(cb2[:], d_cb2.ap()[:])
            nc.sync.dma_start(cb3[:], d_cb3.ap()[:])
            nc.sync.dma_start(idh[:], d_cbh.ap()[:])
            v1f = cb1[:, 0:256]
            pcol = cb1[:, 256:266]
            v1l = cb2[:, 0:256]
            ploc = cb2[:, 256:274]
            id16 = cb2[:, 274:290]
            oner = cb3[:, 0:128]
            prow = cb3[:, 128:256]
            # prow: wv 0:16 | gv 16:32 | bev 32:48 | wg 48:64 | gg 64:80 |
            #       beg 80:96 | wv2 96:112 | wg2 112:128
            onec = pc.tile([128, 1], F32, tag="onec")
            nc.vector.memset(onec[:], 1.0)

            # ---------------- input DMAs (HWDGE, 512KB each) --------------
            res = []
            for g in range(8):
                t = pres.tile([128, 2048], FP16, tag="res")
                nc.sync.dma_start(t[:], d_a1.ap()[:, g * 2048:(g + 1) * 2048])
                res.append(t)

            def a1c(c):
                return res[c // 2][:, (c % 2) * CF:(c % 2) * CF + CF]

            # ============== v-branch elementwise (Pool engine) ============
            # key: vk = v1*wk + bk ; squares for variance
            vkf = pc.tile([128, N * TV], F32, tag="vkf")
            nc.gpsimd.tensor_scalar(vkf[:], v1f[:], pcol[:, 0:1], pcol[:, 1:2],
                                    OP.mult, OP.add)
            scrk = pc.tile([128, N * TV], F32, tag="scrk")
            nc.gpsimd.tensor_tensor(scrk[:], vkf[:], vkf[:], OP.mult)
            va = pc.tile([128, H * N * TV], F32, tag="va")
            for h in range(H):
                nc.gpsimd.tensor_scalar(
                    va[:, h * N * TV:(h + 1) * N * TV], v1f[:],
                    pcol[:, 2 + h:3 + h], pcol[:, 6 + h:7 + h], OP.mult,
                    OP.add)
            scra = pc.tile([128, H * N * TV], F32, tag="scra")
            nc.gpsimd.tensor_tensor(scra[:], va[:], va[:], OP.mult)
            vkl = pc.tile([CL, N * TV], F32, tag="vkl")
            nc.gpsimd.tensor_scalar(vkl[:], v1l[:], ploc[:, 0:1], ploc[:, 1:2],
                                    OP.mult, OP.add)
            val = pc.tile([CL, H * N * TV], F32, tag="val")
            for h in range(H):
                nc.gpsimd.tensor_scalar(
                    val[:, h * N * TV:(h + 1) * N * TV], v1l[:],
                    ploc[:, 4 + h:5 + h], ploc[:, 8 + h:9 + h], OP.mult,
                    OP.add)

            # ============== BN stats c=0..3 (DVE) =========================
            bns = pc.tile([128, CL * 12], F32, tag="bns")

            def bn_pair(c):
                src = a1c(c)
                nc.vector.bn_stats(bns[:, c * 12:c * 12 + 6], src[:, 0:512])
                nc.vector.bn_stats(bns[:, c * 12 + 6:c * 12 + 12],
                                   src[:, 512:1024])

            for c in range(0, 4):
                bn_pair(c)

            # ============== v-branch reductions (DVE) =====================
            ks = pc.tile([128, 8], F32, tag="ks")
            nc.vector.tensor_reduce(
                ks[:, 0:4], vkf[:].rearrange("p (n t) -> p n t", n=N, t=TV),
                axis=AX.X, op=OP.add)
            nc.vector.tensor_reduce(
                ks[:, 4:8], scrk[:].rearrange("p (n t) -> p n t", n=N, t=TV),
                axis=AX.X, op=OP.add)
            asum = pc.tile([128, 8], F32, tag="asum")
            nc.vector.tensor_reduce(
                asum[:, 0:4],
                va[:].rearrange("p (h n t) -> p n h t", h=H, n=N, t=TV),
                axis=AX.XY, op=OP.add)
            nc.vector.tensor_reduce(
                asum[:, 4:8],
                scra[:].rearrange("p (h n t) -> p n h t", h=H, n=N, t=TV),
                axis=AX.XY, op=OP.add)

            for c in range(4, 8):
                bn_pair(c)

            # ------- v-branch cross-partition reduction + finalize --------
            pp_ks = pps.tile([128, 32], F32, tag="sm")
            pp_as = pps.tile([128, 32], F32, tag="sm")
            nc.tensor.matmul(pp_ks[0:1, 0:8], onec[:], ks[:], start=True,
                             stop=True)
            nc.tensor.matmul(pp_as[0:1, 0:8], onec[:], asum[:], start=True,
                             stop=True)
            kr = pc.tile([1, 8], F32, tag="kr")
            ar = pc.tile([1, 8], F32, tag="ar")
            nc.scalar.copy(kr[:], pp_ks[0:1, 0:8])
            nc.scalar.copy(ar[:], pp_as[0:1, 0:8])

            # kv layout 0:4 kmean | 4:8 kex2 | 8:12 kvar | 12:16 km^2
            #           16:20 amean | 20:24 aex2 | 24:28 avar | 28:32 am^2
            kv = pc.tile([1, 32], F32, tag="kv")
            nc.vector.tensor_scalar_mul(kv[:, 0:4], kr[:, 0:4], 1.0 / NKEY)
            nc.vector.tensor_scalar_mul(kv[:, 4:8], kr[:, 4:8], 1.0 / NKEY)
            nc.vector.tensor_tensor(kv[:, 12:16], kv[:, 0:4], kv[:, 0:4],
                                    OP.mult)
            nc.vector.tensor_tensor(kv[:, 8:12], kv[:, 4:8], kv[:, 12:16],
                                    OP.subtract)
            nc.vector.tensor_scalar_mul(kv[:, 16:20], ar[:, 0:4], 1.0 / NATT)
            nc.vector.tensor_scalar_mul(kv[:, 20:24], ar[:, 4:8], 1.0 / NATT)
            nc.vector.tensor_tensor(kv[:, 28:32], kv[:, 16:20], kv[:, 16:20],
                                    OP.mult)
            nc.vector.tensor_tensor(kv[:, 24:28], kv[:, 20:24], kv[:, 28:32],
                                    OP.subtract)

            qv = pc.tile([1, 8], F32, tag="qv")
            nc.vector.tensor_scalar_add(qv[:, 0:4], kv[:, 8:12], GLN_EPS)
            nc.vector.tensor_scalar_add(qv[:, 4:8], kv[:, 24:28], GLN_EPS)
            rsv = _rsqrt_hack(nc, pc, qv[:], 8, "v")  # 0:4 rs_key | 4:8 rs_at

            # b1 row [1,28]: kmean(4) | rs_key(4) | amean*rs_attn(4) |
            #                rs_attn repeated n-major h-minor (16)
            b1 = pc.tile([1, 28], F32, tag="b1")
            nc.vector.tensor_copy(b1[:, 0:4], kv[:, 0:4])
            nc.vector.tensor_copy(b1[:, 4:8], rsv[:, 0:4])
            nc.vector.tensor_tensor(b1[:, 8:12], kv[:, 16:20], rsv[:, 4:8],
                                    OP.mult)
            b1rep = b1[:, 12:28].rearrange("p (n x) -> p n x", n=N, x=H)
            rsat = rsv[:, 4:8].rearrange("p (n x) -> p n x", n=N, x=1)
            for h in range(H):
                nc.vector.tensor_copy(b1rep[:, :, h:h + 1], rsat[:])

            pp_b1 = pps.tile([128, 32], F32, tag="sm")
            nc.tensor.matmul(pp_b1[:, 0:28], oner[:], b1[:], start=True,
                             stop=True)
            bc1 = pc.tile([128, 28], F32, tag="bc1")
            nc.scalar.copy(bc1[:], pp_b1[:, 0:28])

            for c in range(8, 12):
                bn_pair(c)

            # ---------------- local v-branch normalize (DVE) --------------
            # ploc: wk 0 | bk 1 | gk 2 | bek 3 | wa 4:8 | ba 8:12 |
            #       ga/4 12:16 | sum(ga)/4 16 | sum(bea)/4 17
            kscol = pc.tile([CL, N], F32, tag="kscol")
            kbcol = pc.tile([CL, N], F32, tag="kbcol")
            for n in range(N):
                nc.vector.tensor_tensor(kscol[:, n:n + 1], ploc[:, 2:3],
                                        bc1[0:CL, 4 + n:5 + n], OP.mult)
                nc.vector.tensor_tensor(kbcol[:, n:n + 1], kscol[:, n:n + 1],
                                        bc1[0:CL, n:n + 1], OP.mult)
                nc.vector.tensor_tensor(kbcol[:, n:n + 1], ploc[:, 3:4],
                                        kbcol[:, n:n + 1], OP.subtract)
            # vkln/soft stored in (ns, b, tv) column order so the transpose
            # lhsT slice [16, 128] is contiguous: perm(n) = (n%2)*2 + n//2
            perm = [(n % 2) * 2 + n // 2 for n in range(N)]
            vkln = pc.tile([CL, N * TV], F32, tag="vkln")
            for n in range(N):
                nc.vector.tensor_scalar(
                    vkln[:, perm[n] * TV:(perm[n] + 1) * TV],
                    vkl[:, n * TV:(n + 1) * TV],
                    kscol[:, n:n + 1], kbcol[:, n:n + 1], OP.mult, OP.add)

            ga16 = pc.tile([CL, N * H], F32, tag="ga16")
            for n in range(N):
                nc.vector.tensor_copy(ga16[:, n * H:(n + 1) * H],
                                      ploc[:, 12:16])
            sc16 = pc.tile([CL, N * H], F32, tag="sc16")
            nc.vector.tensor_tensor(sc16[:], ga16[:], bc1[0:CL, 12:28],
                                    OP.mult)
            bicol = pc.tile([CL, N], F32, tag="bicol")
            for n in range(N):
                nc.vector.tensor_tensor(bicol[:, n:n + 1], ploc[:, 16:17],
                                        bc1[0:CL, 8 + n:9 + n], OP.mult)
                nc.vector.tensor_tensor(bicol[:, n:n + 1], ploc[:, 17:18],
                                        bicol[:, n:n + 1], OP.subtract)
            # vm[n] = sum_h val[h,n]*sc16[n,h] + bicol[n]  (ga,bea host-/4)
            vm = pc.tile([CL, N * TV], F32, tag="vm")
            for n in range(N):
                dst = vm[:, n * TV:(n + 1) * TV]
                nc.vector.tensor_scalar(
                    dst, val[:, n * TV:n * TV + TV],
                    sc16[:, n * H:n * H + 1], bicol[:, n:n + 1],
                    OP.mult, OP.add)
                for h in range(1, H):
                    nc.vector.scalar_tensor_tensor(
                        dst,
                        val[:, h * N * TV + n * TV:h * N * TV + n * TV + TV],
                        sc16[:, n * H + h:n * H + h + 1], dst, OP.mult,
                        OP.add)
            # softmax over tv per (c, n)
            mx = pc.tile([CL, N], F32, tag="mx")
            nc.vector.tensor_reduce(
                mx[:], vm[:].rearrange("p (n t) -> p n t", n=N, t=TV),
                axis=AX.X, op=OP.max)
            nmx = pc.tile([CL, N], F32, tag="nmx")
            nc.vector.tensor_scalar_mul(nmx[:], mx[:], -1.0)
            ex = pc.tile([CL, N * TV], F32, tag="ex")
            ssum = pc.tile([CL, N], F32, tag="ssum")
            for n in range(N):
                nc.scalar.activation(
                    ex[:, n * TV:(n + 1) * TV], vm[:, n * TV:(n + 1) * TV],
                    AF.Exp, bias=nmx[:, n:n + 1],
                    accum_out=ssum[:, n:n + 1])
            rcp = pc.tile([CL, N], F32, tag="rcp")
            nc.vector.reciprocal(rcp[:], ssum[:])
            soft = pc.tile([CL, N * TV], F32, tag="soft")
            for n in range(N):
                nc.vector.tensor_scalar_mul(
                    soft[:, perm[n] * TV:(perm[n] + 1) * TV],
                    ex[:, n * TV:(n + 1) * TV], rcp[:, n:n + 1])

            for c in range(12, 16):
                bn_pair(c)

            # ---------------- transpose to (b,k) x (ns,c) -----------------
            tkey = pc.tile([128, NS * CL], F32, tag="tkey")
            tatt = pc.tile([128, NS * CL], F32, tag="tatt")
            for (src, dst) in ((vkln, tkey), (soft, tatt)):
                for ns in range(NS):
                    pt = pps.tile([128, 32], F32, tag="sm")
                    nc.tensor.matmul(pt[:, 0:CL],
                                     src[:, ns * B * TV:(ns + 1) * B * TV],
                                     id16[:], start=True, stop=True)
                    nc.scalar.copy(dst[:, ns * CL:(ns + 1) * CL],
                                   pt[:, 0:CL])

            # ============== per-group BN finalize + fused loop ============
            alpha = pc.tile([128, NS * CL], F32, tag="alpha")
            beta = pc.tile([128, NS * CL], F32, tag="beta")
            bcab = pc.tile([128, 64], F32, tag="bcab")
            # bcab row layout per group g (cols g*32..): Av 0:8 | Bv 8:16 |
            #                                            Ag 16:24 | Bg 24:32

            def finalize_group(g):
                c0 = g * GC
                bnsg = bns[:, c0 * 12:(c0 + GC) * 12]
                v4 = bnsg.rearrange("p (c h k) -> p c k h", c=GC, h=4, k=3)
                stk = pc.tile([128, 3 * GC], F32, tag=f"stk{g}")
                # per-partition: sum of the 4 means / 4 cv's / 4 mean^2's
                nc.vector.tensor_reduce(stk[:, 0:GC], v4[:, :, 1:2, :],
                                        axis=AX.X, op=OP.add)
                nc.vector.tensor_reduce(stk[:, GC:2 * GC], v4[:, :, 2:3, :],
                                        axis=AX.X, op=OP.add)
                msq = pc.tile([128, 4 * GC], F32, tag=f"msq{g}")
                mv = msq[:].rearrange("p (c o h) -> p c o h", c=GC, o=1, h=4)
                nc.vector.tensor_tensor(mv[:], v4[:, :, 1:2, :],
                                        v4[:, :, 1:2, :], OP.mult)
                nc.vector.tensor_reduce(stk[:, 2 * GC:3 * GC], mv[:],
                                        axis=AX.X, op=OP.add)
                # cross-partition reduce -> [1, 24]
                pr = pps.tile([128, 32], F32, tag="sm")
                nc.tensor.matmul(pr[0:1, 0:3 * GC], onec[:], stk[:],
                                 start=True, stop=True)
                rr = pc.tile([1, 3 * GC], F32, tag=f"rr{g}")
                nc.scalar.copy(rr[:], pr[0:1, 0:3 * GC])
                # rows: mx = msum/512 ; ex2 = cvsum/NBN + msqsum/512
                mxr = pc.tile([1, 4 * GC], F32, tag=f"mxr{g}")
                # mxr: mx 0:8 | ex2 8:16 | var 16:24 | scratch 24:32
                nc.vector.tensor_scalar_mul(mxr[:, 0:GC], rr[:, 0:GC],
                                            1.0 / 512.0)
                nc.vector.tensor_scalar_mul(mxr[:, 24:32], rr[:, 2 * GC:],
                                            1.0 / 512.0)
                nc.vector.tensor_scalar(mxr[:, 8:16], rr[:, GC:2 * GC],
                                        1.0 / NBN, None, OP.mult)
                nc.vector.tensor_tensor(mxr[:, 8:16], mxr[:, 8:16],
                                        mxr[:, 24:32], OP.add)
                nc.vector.tensor_tensor(mxr[:, 24:32], mxr[:, 0:GC],
                                        mxr[:, 0:GC], OP.mult)
                nc.vector.tensor_tensor(mxr[:, 16:24], mxr[:, 8:16],
                                        mxr[:, 24:32], OP.subtract)
                # qb [1,16]: var*wv2+eps | var*wg2+eps
                qb = pc.tile([1, 2 * GC], F32, tag=f"qb{g}")
                nc.vector.tensor_tensor(qb[:, 0:GC], mxr[:, 16:24],
                                        prow[:, 96 + c0:96 + c0 + GC],
                                        OP.mult)
                nc.vector.tensor_tensor(qb[:, GC:], mxr[:, 16:24],
                                        prow[:, 112 + c0:112 + c0 + GC],
                                        OP.mult)
                nc.vector.tensor_scalar_add(qb[:], qb[:], BN_EPS)
                rsb = _rsqrt_hack(nc, pc, qb[:], 2 * GC, f"b{g}")
                # ab row [1,32]: Av | Bv | Ag | Bg
                ab = pc.tile([1, 32], F32, tag=f"ab{g}")
                nc.vector.tensor_tensor(ab[:, 0:8], rsb[:, 0:8],
                                        prow[:, 16 + c0:16 + c0 + GC],
                                        OP.mult)
                nc.vector.tensor_tensor(ab[:, 0:8], ab[:, 0:8],
                                        prow[:, c0:c0 + GC], OP.mult)
                nc.vector.tensor_tensor(ab[:, 8:16], mxr[:, 0:GC],
                                        ab[:, 0:8], OP.mult)
                nc.vector.tensor_tensor(ab[:, 8:16],
                                        prow[:, 32 + c0:32 + c0 + GC],
                                        ab[:, 8:16], OP.subtract)
                nc.vector.tensor_tensor(ab[:, 16:24], rsb[:, 8:16],
                                        prow[:, 64 + c0:64 + c0 + GC],
                                        OP.mult)
                nc.vector.tensor_tensor(ab[:, 16:24], ab[:, 16:24],
                                        prow[:, 48 + c0:48 + c0 + GC],
                                        OP.mult)
                nc.vector.tensor_tensor(ab[:, 24:32], mxr[:, 0:GC],
                                        ab[:, 16:24], OP.mult)
                nc.vector.tensor_tensor(ab[:, 24:32],
                                        prow[:, 80 + c0:80 + c0 + GC],
                                        ab[:, 24:32], OP.subtract)
                pab = pps.tile([128, 32], F32, tag="sm")
                nc.tensor.matmul(pab[:], oner[:], ab[:], start=True, stop=True)
                bg = bcab[:, g * 32:(g + 1) * 32]
                nc.scalar.copy(bg, pab[:])
                # alpha/beta columns for this group's channels
                for ns in range(NS):
                    asl = slice(ns * CL + c0, ns * CL + c0 + GC)
                    nc.vector.tensor_tensor(alpha[:, asl], tatt[:, asl],
                                            bg[:, 0:8], OP.mult)
                    nc.vector.tensor_tensor(beta[:, asl], tatt[:, asl],
                                            bg[:, 8:16], OP.mult)

            def channel(c):
                g = c // GC
                j = c - g * GC
                bg = bcab[:, g * 32:(g + 1) * 32]
                src = a1c(c)
                # gate = relu(Ag*src + Bg)  (ACT, fp16 out)
                gate = pgate.tile([128, CF], FP16, tag="gate")
                nc.scalar.activation(gate[:], src, AF.Relu,
                                     bias=bg[:, 24 + j:25 + j],
                                     scale=bg[:, 16 + j:17 + j])
                # diagonal weight tiles (DVE, fp16 4x)
                dd = []
                for ns in range(NS):
                    da = pdiag.tile([128, 128], FP16, tag="da")
                    nc.vector.tensor_scalar(
                        da[:], idh[:], alpha[:, ns * CL + c:ns * CL + c + 1],
                        None, OP.mult)
                    dk = pdiag.tile([128, 128], FP16, tag="dk")
                    nc.vector.tensor_scalar(
                        dk[:], idh[:], tkey[:, ns * CL + c:ns * CL + c + 1],
                        None, OP.mult)
                    dd.append((da, dk))
                # PE fuse: P_ns = diag(alpha_ns)@src + diag(key_ns)@gate
                if c % 2 == 0:
                    channel.ost = post.tile([128, 2 * NS * CF], F32,
                                            tag="ost")
                ost = channel.ost
                base = (c % 2) * NS * CF
                for ns in range(NS):
                    da, dk = dd[ns]
                    pt = ppo.tile([128, CF], F32, tag="pfuse")
                    for hh in range(2):
                        sl = slice(hh * 512, (hh + 1) * 512)
                        nc.tensor.matmul(pt[:, sl], da[:], src[:, sl],
                                         start=True, stop=False)
                    for hh in range(2):
                        sl = slice(hh * 512, (hh + 1) * 512)
                        nc.tensor.matmul(pt[:, sl], dk[:], gate[:, sl],
                                         start=False, stop=True)
                    # PSUM -> SBUF copy with beta bias
                    dst = ost[:, base + ns * CF:base + (ns + 1) * CF]
                    bcol = beta[:, ns * CL + c:ns * CL + c + 1]
                    if (ns == 0) != (c in ACT_COPY1):
                        nc.scalar.activation(dst, pt[:], AF.Identity,
                                             bias=bcol, scale=1.0)
                    else:
                        nc.vector.tensor_scalar(dst, pt[:], 1.0, bcol,
                                                OP.mult, OP.add)
                if c % 2 == 1:
                    nc.sync.dma_start(
                        d_out.ap()[:, (c - 1) * NS * CF:(c + 1) * NS * CF],
                        ost[:])

            finalize_group(0)
            for c in range(0, 8):
                channel(c)
            finalize_group(1)
            for c in range(8, 16):
                channel(c)

    nc.compile()
    return nc


_NC_CACHE = None


def _get_nc():
    global _NC_CACHE
    if _NC_CACHE is None:
        _NC_CACHE = _build()
    return _NC_CACHE


def _pack_inputs(a1, v1, w_gate, b_gate, g_gate, be_gate,
                 w_val, b_val, g_val, be_val,
                 w_attn, b_attn, g_attn, be_attn,
                 w_key, b_key, g_key, be_key):
    f32 = np.float32
    a1 = np.asarray(a1, f32)
    v1 = np.asarray(v1, f32)
    # full-channel tensors (replicated)
    v1f = np.ascontiguousarray(v1.transpose(1, 0, 2).reshape(CA, N * TV))
    wa2 = np.asarray(w_attn, f32).reshape(CA, H)
    ba2 = np.asarray(b_attn, f32).reshape(CA, H)
    ga2 = np.asarray(g_attn, f32).reshape(CA, H)
    bea2 = np.asarray(be_attn, f32).reshape(CA, H)
    pcol = np.concatenate(
        [np.asarray(w_key, f32)[:, None], np.asarray(b_key, f32)[:, None],
         wa2, ba2], axis=1)
    cb1 = np.ascontiguousarray(np.concatenate([v1f, pcol], axis=1))
    id16 = np.eye(CL, dtype=f32)
    idh = np.eye(128, dtype=np.float16)

    in_maps = []
    for i in range(NCORE):
        sl = slice(i * CL, (i + 1) * CL)
        x = a1[:, sl].reshape(B, CL, TV, RP, FQ)
        x = np.ascontiguousarray(x.transpose(0, 2, 1, 3, 4))
        a1s = x.reshape(128, AFREE).astype(np.float16)
        v1l = np.ascontiguousarray(
            v1[:, sl].transpose(1, 0, 2).reshape(CL, N * TV))
        ga4 = ga2[sl] * 0.25
        ploc = np.concatenate(
            [np.asarray(w_key, f32)[sl, None],
             np.asarray(b_key, f32)[sl, None],
             np.asarray(g_key, f32)[sl, None],
             np.asarray(be_key, f32)[sl, None],
             wa2[sl], ba2[sl], ga4,
             ga4.sum(1, keepdims=True),
             (bea2[sl] * 0.25).sum(1, keepdims=True)], axis=1)
        cb2 = np.ascontiguousarray(
            np.concatenate([v1l, ploc, id16], axis=1))
        wv = np.asarray(w_val, f32)[sl]
        wg = np.asarray(w_gate, f32)[sl]
        prow = np.concatenate(
            [wv, np.asarray(g_val, f32)[sl],
             np.asarray(be_val, f32)[sl], wg,
             np.asarray(g_gate, f32)[sl],
             np.asarray(be_gate, f32)[sl],
             wv * wv, wg * wg])[None, :]
        cb3 = np.ascontiguousarray(
            np.concatenate([np.ones((1, 128), f32), prow], axis=1))
        in_maps.append({"a1s": a1s, "cb1": cb1, "cb2": cb2, "cb3": cb3,
                        "cbh": idh})
    return in_maps


def _unpack_output(results):
    out = np.empty((N, CA, T, FQ), np.float32)
    for i in range(NCORE):
        r = np.asarray(results[i]["out"]).reshape(B, TV, CL, NS, RP, FQ)
        r = r.transpose(0, 3, 2, 1, 4, 5).reshape(N, CL, T, FQ)
        out[:, i * CL:(i + 1) * CL] = r
    return out


def _install_ntff_shim():
    """The agent image's ``antenv`` lacks ``axon_hooks``; recreate it and
    register the ctypes NTFF hook against /opt/axon/libaxon_pjrt.so (the
    same mechanism trn_boot uses when the module exists)."""
    import sys
    import types
    import ctypes
    import contextlib

    if "antenv.axon_hooks" in sys.modules:
        return True
    so_path = "/opt/axon/libaxon_pjrt.so"
    try:
        lib = ctypes.CDLL(so_path)
    except OSError:
        return False
    if not hasattr(lib, "axon_start_nrt_profile"):
        return False
    lib.axon_start_nrt_profile.argtypes = [ctypes.POINTER(ctypes.c_int64),
                                           ctypes.c_size_t]
    lib.axon_start_nrt_profile.restype = ctypes.c_int64
    lib.axon_stop_nrt_profile.argtypes = [ctypes.c_char_p]
    lib.axon_stop_nrt_profile.restype = ctypes.c_int64

    @contextlib.contextmanager
    def _hook(output_dir, device_ids):
        import jax
        jax.devices()
        if device_ids:
            ids = (ctypes.c_int64 * len(device_ids))(*device_ids)
            rc = lib.axon_start_nrt_profile(ids, len(device_ids))
        else:
            rc = lib.axon_start_nrt_profile(None, 0)
        if rc != 0:
            raise RuntimeError(f"axon_start_nrt_profile rc={rc}")
        try:
            yield
        finally:
            n = lib.axon_stop_nrt_profile(str(output_dir).encode())
            print(f"profile: {n} file(s) written to {output_dir}",
                  file=sys.stderr)

    mod = types.ModuleType("antenv.axon_hooks")
    _state = {"hook": _hook}
    mod.get_axon_ntff_profile_hook = lambda: _state["hook"]

    def set_axon_ntff_profile_hook(h):
        _state["hook"] = h

    mod.set_axon_ntff_profile_hook = set_axon_ntff_profile_hook
    import antenv
    antenv.axon_hooks = mod
    sys.modules["antenv.axon_hooks"] = mod
    return True


def run(inputs, trace=False, **trace_kwargs):
    """Returns (output, BassKernelResults)."""
    nc = _get_nc()
    in_maps = _pack_inputs(**inputs)
    if trace and not _install_ntff_shim():
        trace = False
    br = run_bass_kernel_spmd(nc, in_maps, core_ids=list(range(NCORE)),
                              trace=trace, **trace_kwargs)
    return _unpack_output(br.results), br


def kernel(**inputs):
    out, _ = run(inputs)
    return out
